# revision 16
# baseline (speedup 1.0000x reference)
"""GriffinBlock1D Trainium2 Bass kernel, v2 (bf16).

Sharding: 8 cores = (batch b, T-half). The GLRU gate u=sigmoid(W x) decays
scan contributions below 1e-6 within 32 tokens, so each core cold-starts its
own scan WU=32 tokens early -- no GLRU replication, no cross-core traffic.
Scan covers [h0-48, h0+528) (SL=608); host zero-pads out-of-range tokens
(u=0.5, cand=0 keeps h exactly 0 through half0's warmup).

Layout: feature-major bf16 activations [d(part), t(free)], fp32 PSUM
accumulation, fp32 scan state / LN stat rows / x3+LN4 path. Banded
attention: 96-token query chunks, one 128-wide stationary k-slice and one
token-major v block per chunk; per head all scores land in a single
bank-sized PSUM tile so exp and band-mask are one op each; softmax
denominators ride the e-tiles via ones matmuls; the output projection
accumulates per-head inside the attention loop and the x1 residual is folded
in via an identity matmul. LayerNorm: 1/D-prescaled ones-matmul stats, m^2
via one Act Square read of PSUM, rstd = reciprocal(sqrt(var)) with the sqrt
act table pre-warmed by anchored dummy ops, mean/rstd partition-broadcast on
gpsimd, apply split across DVE (2x bf16) and gpsimd. A short dummy-matmul
stream at t=0 releases the PE HAM clock gate before real work arrives.
"""

import numpy as np
import ml_dtypes

import concourse.bass as bass
import concourse.mybir as mybir
import concourse.tile as tile
from concourse import bacc
from concourse.bass_utils import run_bass_kernel_spmd

F32 = mybir.dt.float32
F32R = mybir.dt.float32r
BF16 = mybir.dt.bfloat16
AF = mybir.ActivationFunctionType
ALU = mybir.AluOpType

B, T, D, H, WIN, FFD = 4, 1024, 512, 4, 16, 2048
DH = D // H              # 128
TL = T // 2              # 512 tokens per core
WU, HALO = 32, 16
SL = WU + HALO + TL + HALO   # 608 scan cols
WT = TL + 2 * HALO           # 544-token attention window
EPS = 1e-5
SCL = 1.0 / np.sqrt(DH)
NCORES = 8
NCH = 6                      # attention chunks: 5x96 + 1x32

_CACHE = {}


def _build_nc(trivial_gb=True):
    nc = bacc.Bacc("TRN2", target_bir_lowering=False, debug=False)

    di = lambda n, s, dt: nc.dram_tensor(n, s, dt, kind="ExternalInput")
    xt_d = di("xt", [D, SL], BF16)
    winT_d = di("w_inT", [D, 2 * D], BF16)
    wstT_d = di("w_stateT", [D, D], BF16)      # pre-negated on host
    wqkT_d = di("wqkT", [D, 2 * D], BF16)      # WqT | WkT
    wvpT_d = di("wvpT", [D, 2 * D], BF16)      # WvT | WpT
    w1T_d = di("w1T", [D, FFD], BF16)
    w2T_d = di("w2T", [FFD, D], BF16)
    lngb_d = di("lngb", [128, 4, 2, 4], F32)   # [p, ln_idx, g/b, et]
    b1_d = di("b1c", [128, FFD // 128], F32)
    b2_d = di("b2c", [128, D // 128], F32)
    mask_d = di("maskc", [128, 512], BF16)
    onesb_d = di("onesb", [128, 2], BF16)      # cols: 1.0, 1/D
    ones32_d = di("ones32", [128, 1], F32R)    # 1/D
    onesr_d = di("onesr", [1, 128], F32R)      # PE-broadcast stationary
    ident_d = di("ident", [128, 128], BF16)    # identity (residual-into-PSUM)
    out_d = nc.dram_tensor("outp", [4, 128, TL], F32, kind="ExternalOutput")

    with tile.TileContext(nc) as tc:
        with tc.tile_pool(name="cp", bufs=1) as cp, \
             tc.tile_pool(name="sq", bufs=3) as sqp, \
             tc.tile_pool(name="rw", bufs=3) as rw, \
             tc.tile_pool(name="bc", bufs=3) as bcp, \
             tc.tile_pool(name="pq", bufs=2, space="PSUM") as pq:

            # ---- input DMAs; wst + odd chunks ride the Act queue so the
            # SP queue serves xt/winT even chunks with minimum latency ----
            xt_sb = cp.tile([128, 4, SL], BF16, tag="xt")
            winT_sb = cp.tile([128, 4, 2 * D], BF16, tag="winT")
            wstT_sb = cp.tile([128, 4, D], BF16, tag="wst")
            nc.scalar.dma_start(wstT_sb,
                                wstT_d[:, :].rearrange("(a p) e -> p a e", p=128))
            for kk in (0, 2):
                nc.sync.dma_start(xt_sb[:, kk, :], xt_d[kk * 128:(kk + 1) * 128, :])
                nc.sync.dma_start(winT_sb[:, kk, :],
                                  winT_d[kk * 128:(kk + 1) * 128, :])
            for kk in (1, 3):
                nc.scalar.dma_start(xt_sb[:, kk, :], xt_d[kk * 128:(kk + 1) * 128, :])
                nc.scalar.dma_start(winT_sb[:, kk, :],
                                    winT_d[kk * 128:(kk + 1) * 128, :])
            wqk_sb = cp.tile([128, 8, D], BF16, tag="wqk")
            nc.sync.dma_start(wqk_sb[:, 0:4, :],
                              wqkT_d[:, 0:D].rearrange("(a p) e -> p a e", p=128))
            nc.sync.dma_start(wqk_sb[:, 4:8, :],
                              wqkT_d[:, D:2 * D].rearrange("(a p) e -> p a e", p=128))
            wvp_sb = cp.tile([128, 8, D], BF16, tag="wvp")
            nc.sync.dma_start(wvp_sb[:, 0:4, :],
                              wvpT_d[:, 0:D].rearrange("(a p) e -> p a e", p=128))
            nc.sync.dma_start(wvp_sb[:, 4:8, :],
                              wvpT_d[:, D:2 * D].rearrange("(a p) e -> p a e", p=128))
            mask_sb = cp.tile([128, 512], BF16, tag="mask")
            nc.sync.dma_start(mask_sb, mask_d[:, :])
            w1all = cp.tile([128, 4, FFD], BF16, tag="w1all")
            for kk in range(4):
                nc.sync.dma_start(w1all[:, kk, :],
                                  w1T_d[kk * 128:(kk + 1) * 128, :])
            w2all = cp.tile([128, 16, D], BF16, tag="w2all")
            for qd in range(4):
                nc.sync.dma_start(
                    w2all[:, qd * 4:(qd + 1) * 4, :],
                    w2T_d[qd * 512:(qd + 1) * 512, :]
                    .rearrange("(c p) e -> p c e", p=128))
            lngb_sb = cp.tile([128, 4, 2, 4], F32, tag="lngb")
            nc.sync.dma_start(lngb_sb, lngb_d[:, :, :, :])
            b1_sb = cp.tile([128, FFD // 128], F32, tag="b1")
            nc.sync.dma_start(b1_sb, b1_d[:, :])
            b2_sb = cp.tile([128, D // 128], F32, tag="b2")
            nc.sync.dma_start(b2_sb, b2_d[:, :])

            onesb_sb = cp.tile([128, 2], BF16, tag="onesb")
            nc.scalar.dma_start(onesb_sb, onesb_d[:, :])
            ones16 = onesb_sb[:, 0:1]
            onesD16 = onesb_sb[:, 1:2]
            onesD32 = cp.tile([128, 1], F32R, tag="onesD32")
            nc.scalar.dma_start(onesD32, ones32_d[:, :])
            onesr = cp.tile([1, 128], F32R, tag="onesr")
            nc.scalar.dma_start(onesr, onesr_d[:, :])
            ident_sb = cp.tile([128, 128], BF16, tag="ident")
            nc.scalar.dma_start(ident_sb, ident_d[:, :])

            warm_s = rw.tile([1, 1], F32, tag="warm", name="warm_sig")
            nc.scalar.activation(warm_s, xt_sb[0:1, 0, 0:1], AF.Sigmoid, bias=1.0, scale=0.0)

            # PE ramp warm-up: the HAM clock gate releases after ~3us of
            # sustained activity; burn idle DMA-wait time so real matmuls
            # start at full clock
            wtile = cp.tile([128, 512], BF16, tag="wtile")
            nc.vector.memset(wtile, 0.0)
            for wi in range(8):
                wp = pq.tile([1, 512], F32, tag="sc", name=f"wp{wi % 2}")
                nc.tensor.matmul(wp, wtile[:, 0:1], wtile, start=True, stop=True)

            u_sb = cp.tile([128, 4, SL], BF16, tag="u")
            w_sb = cp.tile([128, 4, SL], BF16, tag="w")
            gv2_sb = cp.tile([128, 4, SL], BF16, tag="gv2")
            y_sb = cp.tile([128, 4, SL], BF16, tag="y")

            # ---------------- LayerNorm helper (all-bf16 data path) --------
            def layer_norm(xin, ncols, out_get, ln_idx, col_splits=1,
                           post_et=None, f32_out=False, stats_inline=None,
                           bf16_chain=False, chunk=512):
                BDT = F32 if (f32_out and not bf16_chain) else BF16
                """xin(et)->AP [128,ncols] bf16; writes out_get(et) bf16.
                Stats stationary pre-scaled by 1/D; f32 stat rows; rstd via
                Ln+Exp; bf16 broadcasts; apply split DVE(et0-2)/Pool(et3)."""
                stps = []
                for ci in range((ncols + chunk - 1) // chunk):
                    c0 = ci * chunk
                    cn = min(chunk, ncols - c0)
                    s1p = pq.tile([1, cn], F32, tag="sc", name=f"s1p{ln_idx}_{ci}")
                    s2p = pq.tile([1, cn], F32, tag="sc", name=f"s2p{ln_idx}_{ci}")
                    stps.append((c0, cn, s1p, s2p))

                def stats_one(ci, et):
                    c0, cn, s1p, s2p = stps[ci]
                    x_ap = xin(et)[:, c0:c0 + cn]
                    if f32_out:
                        # xin is an F32R tile; DVE reads it as F32 bits
                        sq = sqp.tile([128, cn], F32R, tag="sq")
                        if et % 2 == 1:
                            nc.vector.tensor_mul(sq, x_ap.bitcast(F32),
                                                 x_ap.bitcast(F32))
                        else:
                            nc.gpsimd.tensor_mul(sq, x_ap.bitcast(F32),
                                                 x_ap.bitcast(F32))
                        nc.tensor.matmul(s1p, onesD32, x_ap,
                                         start=et == 0, stop=et == 3)
                        nc.tensor.matmul(s2p, onesD32, sq,
                                         start=et == 0, stop=et == 3)
                    else:
                        sq = sqp.tile([128, cn], BF16, tag="sq")
                        if et % 2 == 1:
                            nc.vector.tensor_mul(sq, x_ap, x_ap)
                        else:
                            nc.gpsimd.tensor_mul(sq, x_ap, x_ap)
                        nc.tensor.matmul(s1p, onesD16, x_ap,
                                         start=et == 0, stop=et == 3)
                        nc.tensor.matmul(s2p, onesD16, sq,
                                         start=et == 0, stop=et == 3)

                if stats_inline is None:
                    for ci in range(len(stps)):
                        for et in range(4):
                            stats_one(ci, et)
                else:
                    assert len(stps) == 1
                    stats_inline(lambda et: stats_one(0, et))
                # rows (f32): mm = m^2, vr = (s2/D + EPS) - mm
                width = ncols // col_splits
                bcs = []
                for hf in range(col_splits):
                    h0c = hf * width
                    mm = rw.tile([1, width], F32, tag="mm2", name=f"mm{ln_idx}_{hf}")
                    vr = rw.tile([1, width], F32, tag="vr", name=f"vr{ln_idx}_{hf}")
                    mrow = rw.tile([1, width], BDT, tag="mrow",
                                   name=f"mrow{ln_idx}_{hf}")
                    for c0, cn, s1p, s2p in stps:
                        lo, hi = max(c0, h0c), min(c0 + cn, h0c + width)
                        if lo >= hi:
                            continue
                        ds_ = slice(lo - h0c, hi - h0c)
                        ss = slice(lo - c0, hi - c0)
                        nc.scalar.activation(mm[:, ds_], s1p[:, ss], AF.Square)
                        nc.scalar.activation(mrow[:, ds_], s1p[:, ss], AF.Copy)
                        nc.vector.scalar_tensor_tensor(
                            vr[:, ds_], s2p[:, ss], EPS, mm[:, ds_],
                            ALU.add, ALU.subtract)
                    sd = rw.tile([1, width], BDT, tag="lnv", name=f"sd{ln_idx}_{hf}")
                    nc.scalar.activation(sd, vr, AF.Sqrt)
                    rs = rw.tile([1, width], BDT, tag="rs", name=f"rs{ln_idx}_{hf}")
                    with nc.allow_low_precision("bf16 rstd row"):
                        nc.vector.reciprocal(rs, sd)
                    mbt = bcp.tile([128, width], BDT, tag="mb")
                    nc.gpsimd.partition_broadcast(mbt, mrow)
                    rbt = bcp.tile([128, width], BDT, tag="rb")
                    nc.gpsimd.partition_broadcast(rbt, rs)
                    bcs.append((slice(h0c, h0c + width), mbt, rbt))
                for et in (3, 0, 2, 1):
                    eng = nc.gpsimd if et in (2, 3) else nc.vector
                    for hs, mbt, rbt in bcs:
                        o = out_get(et)[:, hs]
                        t1 = sqp.tile([128, hs.stop - hs.start], BDT, tag="t1")
                        x_in = xin(et)[:, hs]
                        if f32_out:
                            x_in = x_in.bitcast(F32)
                        eng.tensor_sub(t1, x_in, mbt)
                        if trivial_gb:
                            eng.tensor_mul(o, t1, rbt)
                        else:
                            eng.tensor_mul(t1, t1, rbt)
                            eng.tensor_scalar(
                                o, t1, lngb_sb[:, ln_idx, 0, et],
                                lngb_sb[:, ln_idx, 1, et], ALU.mult, ALU.add)
                    if post_et is not None:
                        post_et(et)

            # ---------------- GLRU ----------------
            CH = [(0, 512), (512, SL - 512)]
            x1 = cp.tile([128, 4, WT], BF16, tag="x1")
            with tc.tile_pool(name="psg", bufs=6, space="PSUM") as psg:
                # gv2 first; kk-outer on the 512-chunk so the first matmuls
                # need only the first xt/winT DMA chunks
                gts = [psg.tile([128, 512], F32, tag="mm", name=f"gv{et}")
                       for et in range(4)]
                for kk in range(4):
                    for et in range(4):
                        nc.tensor.matmul(
                            gts[et], winT_sb[:, kk, D + et * 128:D + (et + 1) * 128],
                            xt_sb[:, kk, 0:512], start=kk == 0, stop=kk == 3)
                for et in range(4):
                    nc.vector.tensor_copy(gv2_sb[:, et, 0:512], gts[et])
                for et in range(4):
                    g = psg.tile([128, SL - 512], F32, tag="mm")
                    for kk in range(4):
                        nc.tensor.matmul(
                            g, winT_sb[:, kk, D + et * 128:D + (et + 1) * 128],
                            xt_sb[:, kk, 512:SL], start=kk == 0, stop=kk == 3)
                    nc.vector.tensor_copy(gv2_sb[:, et, 512:SL], g)
                for et in range(4):                     # u, cand, w, scan per et
                    for c0, cn in CH:
                        g = psg.tile([128, cn], F32, tag="mm")
                        for kk in range(4):
                            nc.tensor.matmul(
                                g, winT_sb[:, kk, et * 128:(et + 1) * 128],
                                xt_sb[:, kk, c0:c0 + cn], start=kk == 0, stop=kk == 3)
                        nc.scalar.activation(u_sb[:, et, c0:c0 + cn], g, AF.Sigmoid)
                    for c0, cn in CH:
                        cd = psg.tile([128, cn], F32, tag="mm")
                        for kk in range(4):
                            nc.tensor.matmul(
                                cd, wstT_sb[:, kk, et * 128:(et + 1) * 128],
                                gv2_sb[:, kk, c0:c0 + cn], start=kk == 0, stop=kk == 3)
                        nc.vector.scalar_tensor_tensor(
                            w_sb[:, et, c0:c0 + cn], u_sb[:, et, c0:c0 + cn], 1.0,
                            cd, ALU.subtract, ALU.mult)
                    nc.vector.tensor_tensor_scan(
                        y_sb[:, et, :], u_sb[:, et, :], w_sb[:, et, :], 0.0,
                        ALU.mult, ALU.add)
                    if et == 3:
                        # pre-warm the sqrt act table (anchored post-sigmoid)
                        warm = rw.tile([1, 1], F32, tag="warm", name="warm1")
                        nc.scalar.activation(warm, u_sb[0:1, 3, SL - 1:SL],
                                             AF.Sqrt, bias=1.0, scale=0.0)

                # ---- LN1: x1 = LN(y[WU:WU+WT]) ----
                layer_norm(lambda et: y_sb[:, et, WU:WU + WT], WT,
                           lambda et: x1[:, et, :], 0)

                # ---- q/k/v ----
                q_sb = cp.tile([128, 4, TL], BF16, tag="q")
                k_sb = cp.tile([128, 4, WT], BF16, tag="k")
                for h in range(4):
                    qp = psg.tile([128, TL], F32, tag="mm")
                    for kk in range(4):
                        nc.tensor.matmul(qp, wqk_sb[:, kk, h * 128:(h + 1) * 128],
                                         x1[:, kk, HALO:HALO + TL],
                                         start=kk == 0, stop=kk == 3)
                    nc.vector.tensor_copy(q_sb[:, h, :], qp)
                    kp = psg.tile([128, 512], F32, tag="mm")
                    kp2 = pq.tile([128, 32], F32, tag="sc")
                    for kk in range(4):
                        nc.tensor.matmul(kp, wqk_sb[:, 4 + kk, h * 128:(h + 1) * 128],
                                         x1[:, kk, 0:512], start=kk == 0, stop=kk == 3)
                        nc.tensor.matmul(kp2, wqk_sb[:, 4 + kk, h * 128:(h + 1) * 128],
                                         x1[:, kk, 512:WT], start=kk == 0, stop=kk == 3)
                    nc.scalar.activation(k_sb[:, h, 0:512], kp, AF.Copy)
                    nc.vector.tensor_copy(k_sb[:, h, 512:WT], kp2)

                # v token-major, one 128-row window per attention chunk
                v_sb = cp.tile([128, NCH, D], BF16, tag="v")
                for c in range(NCH):
                    rows = min(128, WT - 96 * c)
                    vp = psg.tile([128, D], F32, tag="mm")
                    for kk in range(4):
                        nc.tensor.matmul(vp[0:rows, :],
                                         x1[:, kk, 96 * c:96 * c + rows],
                                         wvp_sb[:, kk, :],
                                         start=kk == 0, stop=kk == 3)
                    eng_v = nc.vector if c % 2 else nc.scalar
                    if c % 2:
                        nc.vector.tensor_copy(v_sb[0:rows, c, :], vp[0:rows, :])
                    else:
                        nc.scalar.activation(v_sb[0:rows, c, :], vp[0:rows, :],
                                             AF.Copy)

            # ---------------- banded attention ----------------
            # phase 1: all scores -> e; one 512-wide PSUM tile per head
            warm_e = rw.tile([1, 1], F32, tag="warm", name="warm_exp")
            nc.scalar.activation(warm_e, x1[0:1, 1, 0:1], AF.Exp, bias=1.0, scale=0.0)
            e_all = cp.tile([128, 4, 512], BF16, tag="eall")
            for h in range(4):
                sc = pq.tile([128, 512], F32, tag="sc")
                for c in range(5):
                    nc.tensor.matmul(sc[:, 96 * c:96 * c + 96],
                                     k_sb[:, h, 96 * c:96 * c + 128],
                                     q_sb[:, h, 96 * c:96 * c + 96],
                                     start=True, stop=True)
                nc.tensor.matmul(sc[0:64, 480:512], k_sb[:, h, 480:WT],
                                 q_sb[:, h, 480:TL], start=True, stop=True)
                nc.tensor.matmul(sc[64:128, 480:512], wtile[:, 0:64],
                                 wtile[:, 0:32], start=True, stop=True)
                nc.scalar.activation(e_all[:, h, :], sc, AF.Exp, scale=SCL)
                nc.gpsimd.tensor_mul(e_all[:, h, :], e_all[:, h, :], mask_sb)
                if h == 3:
                    warm3 = rw.tile([1, 1], F32, tag="warm", name="warm3")
                    nc.scalar.activation(warm3, e_all[0:1, 3, 0:1], AF.Sqrt,
                                         bias=1.0, scale=0.0)

            # phase 2: den + ao per head, proj accumulation interleaved
            a2 = cp.tile([128, 4, TL], BF16, tag="a2")
            x2pre = cp.tile([128, 4, TL], BF16, tag="x2pre")
            x2 = cp.tile([128, 4, TL], BF16, tag="x2")
            x3 = cp.tile([128, 4, TL], F32R, tag="x3")
            outt = cp.tile([128, 4, TL], F32, tag="outt")
            with tc.tile_pool(name="psf", bufs=4, space="PSUM") as psf:
                proj = [psf.tile([128, TL], F32, tag="mm", name=f"proj{i}")
                        for i in range(4)]
                with tc.tile_pool(name="pa", bufs=2, space="PSUM") as pa:
                    for h in range(4):
                        hsl = slice(h * 128, (h + 1) * 128)
                        den = pq.tile([1, TL], F32, tag="sc", name=f"den{h}")
                        ao = pa.tile([128, TL], F32, tag="ao")
                        nc.tensor.matmul(den[0:1, 0:480], ones16,
                                         e_all[:, h, 0:480], start=True, stop=True)
                        nc.tensor.matmul(den[0:1, 480:TL], ones16[0:64, :],
                                         e_all[0:64, h, 480:512], start=True, stop=True)
                        for c in range(NCH):
                            tn = 96 if c < 5 else TL - 480
                            kn = min(128, WT - 96 * c)
                            nc.tensor.matmul(ao[:, 96 * c:96 * c + tn],
                                             v_sb[0:kn, c, hsl],
                                             e_all[0:kn, h, 96 * c:96 * c + tn],
                                             start=True, stop=True)
                        rec = rw.tile([1, TL], F32, tag="rec")
                        nc.vector.reciprocal(rec, den)
                        dbc = bcp.tile([128, TL], F32, tag="dbc")
                        nc.gpsimd.partition_broadcast(dbc, rec)
                        nc.vector.tensor_mul(a2[:, h, :], ao, dbc)
                        for et in range(4):
                            nc.tensor.matmul(proj[et],
                                             wvp_sb[:, 4 + h, et * 128:(et + 1) * 128],
                                             a2[:, h, :], start=h == 0, stop=False)
                for et in range(4):
                    nc.tensor.matmul(proj[et], ident_sb,
                                     x1[:, et, HALO:HALO + TL],
                                     start=False, stop=True)
                    if et % 2:
                        nc.vector.tensor_copy(x2pre[:, et, :], proj[et])
                    else:
                        nc.scalar.activation(x2pre[:, et, :], proj[et], AF.Copy)
                layer_norm(lambda et: x2pre[:, et, :], TL,
                           lambda et: x2[:, et, :], 1)

                if trivial_gb:
                    xf = x2
                else:
                    xf = cp.tile([128, 4, TL], BF16, tag="xf")
                    layer_norm(lambda et: x2[:, et, :], TL,
                               lambda et: xf[:, et, :], 2)

                warm_g = rw.tile([1, 1], F32, tag="warm", name="warm_gelu")
                nc.scalar.activation(warm_g, x2[0:1, 1, 0:1], AF.Gelu, bias=1.0, scale=0.0)
                hg = cp.tile([128, 16, TL], BF16, tag="hg")
                for ft in range(16):
                    hp = psf.tile([128, TL], F32, tag="mm")
                    for kk in range(4):
                        nc.tensor.matmul(hp, w1all[:, kk, ft * 128:(ft + 1) * 128],
                                         xf[:, kk, :], start=kk == 0, stop=kk == 3)
                    nc.scalar.activation(hg[:, ft, :], hp, AF.Gelu,
                                         bias=b1_sb[:, ft:ft + 1])
                    if ft == 15:
                        warm2 = rw.tile([1, 1], F32, tag="warm", name="warm2")
                        nc.scalar.activation(warm2, hg[0:1, 15, 0:1], AF.Sqrt, bias=1.0, scale=0.0)

                # FFN2 et-outer; LN4 stats emitted right after each x3-et so
                # only et3's stats trail the last accumulation
                for et in range(4):
                    op = psf.tile([128, TL], F32, tag="mm")
                    for kk in range(16):
                        nc.tensor.matmul(op, w2all[:, kk, et * 128:(et + 1) * 128],
                                         hg[:, kk, :], start=kk == 0, stop=kk == 15)
                    nc.vector.scalar_tensor_tensor(
                        x3[:, et, :], op, b2_sb[:, et:et + 1],
                        x2[:, et, :], ALU.add, ALU.add)

                # ---- LN4 -> output (column-split tail) ----
                layer_norm(lambda et: x3[:, et, :], TL,
                           lambda et: outt[:, et, :], 3, col_splits=2,
                           post_et=lambda et: (nc.scalar if et % 2 else nc.sync)
                           .dma_start(out_d[et, :, :], outt[:, et, :]),
                           f32_out=True)

    nc.compile()
    return nc


def _host_inputs(x, W_in, W_state, glru_g, glru_b, Wq, Wk, Wv, Wp, attn_g,
                 attn_b, ffn_g, ffn_b, W1, b1, W2, b2, out_g, out_b):
    f32, bf = np.float32, ml_dtypes.bfloat16
    cb = lambda a: np.ascontiguousarray(np.asarray(a, f32).astype(bf))
    lngb = np.zeros((128, 4, 2, 4), f32)
    for li, (g, b) in enumerate([(glru_g, glru_b), (attn_g, attn_b),
                                 (ffn_g, ffn_b), (out_g, out_b)]):
        lngb[:, li, 0, :] = np.asarray(g, f32).reshape(4, 128).T
        lngb[:, li, 1, :] = np.asarray(b, f32).reshape(4, 128).T
    shared = {
        "w_inT": cb(np.asarray(W_in).T),
        "w_stateT": cb(-np.asarray(W_state).T),
        "wqkT": cb(np.concatenate([np.asarray(Wq).T, np.asarray(Wk).T], 1)),
        "wvpT": cb(np.concatenate([np.asarray(Wv).T, np.asarray(Wp).T], 1)),
        "w1T": cb(np.asarray(W1).T), "w2T": cb(np.asarray(W2).T),
        "lngb": lngb,
        "b1c": np.ascontiguousarray(np.asarray(b1, f32).reshape(FFD // 128, 128).T),
        "b2c": np.ascontiguousarray(np.asarray(b2, f32).reshape(D // 128, 128).T),
        "onesb": np.broadcast_to(np.array([1.0, 1.0 / D], f32), (128, 2))
                  .astype(bf).copy(),
        "ones32": np.full((128, 1), 1.0 / D, f32),
        "onesr": np.ones((1, 128), f32),
        "ident": np.eye(128, dtype=f32).astype(bf),
    }
    xpad = np.zeros((B, T + WU + HALO + HALO, D), f32)
    xpad[:, WU + HALO:WU + HALO + T] = np.asarray(x, f32)
    in_maps = []
    for core in range(NCORES):
        b, half = core // 2, core % 2
        h0 = half * TL
        m = dict(shared)
        m["xt"] = cb(xpad[b, h0:h0 + SL].T)
        mask = np.zeros((128, 512), f32)
        ii = np.arange(128)[:, None]
        for c in range(NCH):
            tn = 96 if c < 5 else TL - 480
            kn = min(128, WT - 96 * c)
            jj = np.arange(tn)[None, :]
            tp = h0 + 96 * c + ii - HALO        # absolute t' token
            band = (np.abs(ii - jj - HALO) <= WIN) & (tp >= 0) & (tp < T) \
                & (ii < kn)
            mask[:, 96 * c:96 * c + tn] = band
        m["maskc"] = mask.astype(bf)
        in_maps.append(m)
    return in_maps


def kernel(**inputs):
    trivial = all(
        bool(np.all(np.asarray(inputs[g]) == 1.0)) for g in
        ("glru_g", "attn_g", "ffn_g", "out_g")
    ) and all(
        bool(np.all(np.asarray(inputs[b]) == 0.0)) for b in
        ("glru_b", "attn_b", "ffn_b", "out_b")
    )
    key = f"nc_{trivial}"
    if key not in _CACHE:
        _CACHE[key] = _build_nc(trivial_gb=trivial)
    nc = _CACHE[key]
    in_maps = _host_inputs(**inputs)
    res = run_bass_kernel_spmd(nc, in_maps, core_ids=list(range(NCORES)),
                               **_CACHE.get("run_kwargs", {}))
    _CACHE["last_result"] = res
    out = np.empty((B, T, D), np.float32)
    for core in range(NCORES):
        b, half = core // 2, core % 2
        o = np.asarray(res.results[core]["outp"]).astype(np.float32)
        out[b, half * TL:(half + 1) * TL, :] = o.reshape(D, TL).T
    return out


# revision 17
# speedup vs baseline: 1.0013x; 1.0013x over previous
"""GriffinBlock1D Trainium2 Bass kernel, v2 (bf16).

Sharding: 8 cores = (batch b, T-half). The GLRU gate u=sigmoid(W x) decays
scan contributions below 1e-6 within 32 tokens, so each core cold-starts its
own scan WU=32 tokens early -- no GLRU replication, no cross-core traffic.
Scan covers [h0-48, h0+528) (SL=608); host zero-pads out-of-range tokens
(u=0.5, cand=0 keeps h exactly 0 through half0's warmup).

Layout: feature-major bf16 activations [d(part), t(free)], fp32 PSUM
accumulation, fp32 scan state / LN stat rows / x3+LN4 path. Banded
attention: 96-token query chunks, one 128-wide stationary k-slice and one
token-major v block per chunk; per head all scores land in a single
bank-sized PSUM tile so exp and band-mask are one op each; softmax
denominators ride the e-tiles via ones matmuls; the output projection
accumulates per-head inside the attention loop and the x1 residual is folded
in via an identity matmul. LayerNorm: 1/D-prescaled ones-matmul stats, m^2
via one Act Square read of PSUM, rstd = reciprocal(sqrt(var)) with the sqrt
act table pre-warmed by anchored dummy ops, mean/rstd partition-broadcast on
gpsimd, apply split across DVE (2x bf16) and gpsimd. A short dummy-matmul
stream at t=0 releases the PE HAM clock gate before real work arrives.
"""

import numpy as np
import ml_dtypes

import concourse.bass as bass
import concourse.mybir as mybir
import concourse.tile as tile
from concourse import bacc
from concourse.bass_utils import run_bass_kernel_spmd

F32 = mybir.dt.float32
F32R = mybir.dt.float32r
BF16 = mybir.dt.bfloat16
AF = mybir.ActivationFunctionType
ALU = mybir.AluOpType

B, T, D, H, WIN, FFD = 4, 1024, 512, 4, 16, 2048
DH = D // H              # 128
TL = T // 2              # 512 tokens per core
WU, HALO = 32, 16
SL = WU + HALO + TL + HALO   # 608 scan cols
WT = TL + 2 * HALO           # 544-token attention window
EPS = 1e-5
SCL = 1.0 / np.sqrt(DH)
NCORES = 8
NCH = 6                      # attention chunks: 5x96 + 1x32

_CACHE = {}


def _build_nc(trivial_gb=True):
    nc = bacc.Bacc("TRN2", target_bir_lowering=False, debug=False)

    di = lambda n, s, dt: nc.dram_tensor(n, s, dt, kind="ExternalInput")
    xt_d = di("xt", [D, SL], BF16)
    winT_d = di("w_inT", [D, 2 * D], BF16)
    wstT_d = di("w_stateT", [D, D], BF16)      # pre-negated on host
    wqkT_d = di("wqkT", [D, 2 * D], BF16)      # WqT | WkT
    wvpT_d = di("wvpT", [D, 2 * D], BF16)      # WvT | WpT
    w1T_d = di("w1T", [D, FFD], BF16)
    w2T_d = di("w2T", [FFD, D], BF16)
    lngb_d = di("lngb", [128, 4, 2, 4], F32)   # [p, ln_idx, g/b, et]
    b1_d = di("b1c", [128, FFD // 128], F32)
    b2_d = di("b2c", [128, D // 128], F32)
    mask_d = di("maskc", [128, 512], BF16)
    onesb_d = di("onesb", [128, 2], BF16)      # cols: 1.0, 1/D
    ones32_d = di("ones32", [128, 1], F32R)    # 1/D
    onesr_d = di("onesr", [1, 128], F32R)      # PE-broadcast stationary
    ident_d = di("ident", [128, 128], BF16)    # identity (residual-into-PSUM)
    out_d = nc.dram_tensor("outp", [4, 128, TL], F32, kind="ExternalOutput")

    with tile.TileContext(nc) as tc:
        with tc.tile_pool(name="cp", bufs=1) as cp, \
             tc.tile_pool(name="sq", bufs=3) as sqp, \
             tc.tile_pool(name="rw", bufs=3) as rw, \
             tc.tile_pool(name="bc", bufs=3) as bcp, \
             tc.tile_pool(name="pq", bufs=2, space="PSUM") as pq:

            # ---- input DMAs; wst + odd chunks ride the Act queue so the
            # SP queue serves xt/winT even chunks with minimum latency ----
            xt_sb = cp.tile([128, 4, SL], BF16, tag="xt")
            winT_sb = cp.tile([128, 4, 2 * D], BF16, tag="winT")
            wstT_sb = cp.tile([128, 4, D], BF16, tag="wst")
            nc.scalar.dma_start(wstT_sb,
                                wstT_d[:, :].rearrange("(a p) e -> p a e", p=128))
            for kk in (0, 2):
                nc.sync.dma_start(xt_sb[:, kk, :], xt_d[kk * 128:(kk + 1) * 128, :])
                nc.sync.dma_start(winT_sb[:, kk, :],
                                  winT_d[kk * 128:(kk + 1) * 128, :])
            for kk in (1, 3):
                nc.scalar.dma_start(xt_sb[:, kk, :], xt_d[kk * 128:(kk + 1) * 128, :])
                nc.scalar.dma_start(winT_sb[:, kk, :],
                                    winT_d[kk * 128:(kk + 1) * 128, :])
            wqk_sb = cp.tile([128, 8, D], BF16, tag="wqk")
            nc.sync.dma_start(wqk_sb[:, 0:4, :],
                              wqkT_d[:, 0:D].rearrange("(a p) e -> p a e", p=128))
            nc.sync.dma_start(wqk_sb[:, 4:8, :],
                              wqkT_d[:, D:2 * D].rearrange("(a p) e -> p a e", p=128))
            wvp_sb = cp.tile([128, 8, D], BF16, tag="wvp")
            nc.sync.dma_start(wvp_sb[:, 0:4, :],
                              wvpT_d[:, 0:D].rearrange("(a p) e -> p a e", p=128))
            nc.sync.dma_start(wvp_sb[:, 4:8, :],
                              wvpT_d[:, D:2 * D].rearrange("(a p) e -> p a e", p=128))
            mask_sb = cp.tile([128, 512], BF16, tag="mask")
            nc.sync.dma_start(mask_sb, mask_d[:, :])
            w1all = cp.tile([128, 4, FFD], BF16, tag="w1all")
            for kk in range(4):
                nc.sync.dma_start(w1all[:, kk, :],
                                  w1T_d[kk * 128:(kk + 1) * 128, :])
            w2all = cp.tile([128, 16, D], BF16, tag="w2all")
            for qd in range(4):
                nc.sync.dma_start(
                    w2all[:, qd * 4:(qd + 1) * 4, :],
                    w2T_d[qd * 512:(qd + 1) * 512, :]
                    .rearrange("(c p) e -> p c e", p=128))
            lngb_sb = cp.tile([128, 4, 2, 4], F32, tag="lngb")
            nc.sync.dma_start(lngb_sb, lngb_d[:, :, :, :])
            b1_sb = cp.tile([128, FFD // 128], F32, tag="b1")
            nc.sync.dma_start(b1_sb, b1_d[:, :])
            b2_sb = cp.tile([128, D // 128], F32, tag="b2")
            nc.sync.dma_start(b2_sb, b2_d[:, :])

            onesb_sb = cp.tile([128, 2], BF16, tag="onesb")
            nc.scalar.dma_start(onesb_sb, onesb_d[:, :])
            ones16 = onesb_sb[:, 0:1]
            onesD16 = onesb_sb[:, 1:2]
            onesD32 = cp.tile([128, 1], F32R, tag="onesD32")
            nc.scalar.dma_start(onesD32, ones32_d[:, :])
            onesr = cp.tile([1, 128], F32R, tag="onesr")
            nc.scalar.dma_start(onesr, onesr_d[:, :])
            ident_sb = cp.tile([128, 128], BF16, tag="ident")
            nc.scalar.dma_start(ident_sb, ident_d[:, :])

            warm_s = rw.tile([1, 1], F32, tag="warm", name="warm_sig")
            nc.scalar.activation(warm_s, xt_sb[0:1, 0, 0:1], AF.Sigmoid, bias=1.0, scale=0.0)

            # PE ramp warm-up: the HAM clock gate releases after ~3us of
            # sustained activity; burn idle DMA-wait time so real matmuls
            # start at full clock
            wtile = cp.tile([128, 512], BF16, tag="wtile")
            nc.vector.memset(wtile, 0.0)
            for wi in range(8):
                wp = pq.tile([1, 512], F32, tag="sc", name=f"wp{wi % 2}")
                nc.tensor.matmul(wp, wtile[:, 0:1], wtile, start=True, stop=True)

            u_sb = cp.tile([128, 4, SL], BF16, tag="u")
            w_sb = cp.tile([128, 4, SL], BF16, tag="w")
            gv2_sb = cp.tile([128, 4, SL], BF16, tag="gv2")
            y_sb = cp.tile([128, 4, SL], BF16, tag="y")

            # ---------------- LayerNorm helper (all-bf16 data path) --------
            def layer_norm(xin, ncols, out_get, ln_idx, col_splits=1,
                           post_et=None, f32_out=False, stats_inline=None,
                           bf16_chain=False, chunk=512):
                BDT = F32 if (f32_out and not bf16_chain) else BF16
                """xin(et)->AP [128,ncols] bf16; writes out_get(et) bf16.
                Stats stationary pre-scaled by 1/D; f32 stat rows; rstd via
                Ln+Exp; bf16 broadcasts; apply split DVE(et0-2)/Pool(et3)."""
                stps = []
                for ci in range((ncols + chunk - 1) // chunk):
                    c0 = ci * chunk
                    cn = min(chunk, ncols - c0)
                    s1p = pq.tile([1, cn], F32, tag="sc", name=f"s1p{ln_idx}_{ci}")
                    s2p = pq.tile([1, cn], F32, tag="sc", name=f"s2p{ln_idx}_{ci}")
                    stps.append((c0, cn, s1p, s2p))

                def stats_one(ci, et):
                    c0, cn, s1p, s2p = stps[ci]
                    x_ap = xin(et)[:, c0:c0 + cn]
                    if f32_out:
                        # xin is an F32R tile; DVE reads it as F32 bits
                        sq = sqp.tile([128, cn], F32R, tag="sq")
                        if et % 2 == 1:
                            nc.vector.tensor_mul(sq, x_ap.bitcast(F32),
                                                 x_ap.bitcast(F32))
                        else:
                            nc.gpsimd.tensor_mul(sq, x_ap.bitcast(F32),
                                                 x_ap.bitcast(F32))
                        nc.tensor.matmul(s1p, onesD32, x_ap,
                                         start=et == 0, stop=et == 3)
                        nc.tensor.matmul(s2p, onesD32, sq,
                                         start=et == 0, stop=et == 3)
                    else:
                        sq = sqp.tile([128, cn], BF16, tag="sq")
                        if et % 2 == 1:
                            nc.vector.tensor_mul(sq, x_ap, x_ap)
                        else:
                            nc.gpsimd.tensor_mul(sq, x_ap, x_ap)
                        nc.tensor.matmul(s1p, onesD16, x_ap,
                                         start=et == 0, stop=et == 3)
                        nc.tensor.matmul(s2p, onesD16, sq,
                                         start=et == 0, stop=et == 3)

                if stats_inline is None:
                    for ci in range(len(stps)):
                        for et in range(4):
                            stats_one(ci, et)
                else:
                    assert len(stps) == 1
                    stats_inline(lambda et: stats_one(0, et))
                # rows (f32): mm = m^2, vr = (s2/D + EPS) - mm
                width = ncols // col_splits
                bcs = []
                for hf in range(col_splits):
                    h0c = hf * width
                    mm = rw.tile([1, width], F32, tag="mm2", name=f"mm{ln_idx}_{hf}")
                    vr = rw.tile([1, width], F32, tag="vr", name=f"vr{ln_idx}_{hf}")
                    mrow = rw.tile([1, width], BDT, tag="mrow",
                                   name=f"mrow{ln_idx}_{hf}")
                    for c0, cn, s1p, s2p in stps:
                        lo, hi = max(c0, h0c), min(c0 + cn, h0c + width)
                        if lo >= hi:
                            continue
                        ds_ = slice(lo - h0c, hi - h0c)
                        ss = slice(lo - c0, hi - c0)
                        nc.scalar.activation(mm[:, ds_], s1p[:, ss], AF.Square)
                        nc.scalar.activation(mrow[:, ds_], s1p[:, ss], AF.Copy)
                        nc.vector.scalar_tensor_tensor(
                            vr[:, ds_], s2p[:, ss], EPS, mm[:, ds_],
                            ALU.add, ALU.subtract)
                    sd = rw.tile([1, width], BDT, tag="lnv", name=f"sd{ln_idx}_{hf}")
                    nc.scalar.activation(sd, vr, AF.Sqrt)
                    rs = rw.tile([1, width], BDT, tag="rs", name=f"rs{ln_idx}_{hf}")
                    with nc.allow_low_precision("bf16 rstd row"):
                        nc.vector.reciprocal(rs, sd)
                    mbt = bcp.tile([128, width], BDT, tag="mb")
                    nc.gpsimd.partition_broadcast(mbt, mrow)
                    rbt = bcp.tile([128, width], BDT, tag="rb")
                    nc.gpsimd.partition_broadcast(rbt, rs)
                    bcs.append((slice(h0c, h0c + width), mbt, rbt))
                for et in (3, 0, 2, 1):
                    eng = nc.gpsimd if et in (2, 3) else nc.vector
                    for hs, mbt, rbt in bcs:
                        o = out_get(et)[:, hs]
                        t1 = sqp.tile([128, hs.stop - hs.start], BDT, tag="t1")
                        x_in = xin(et)[:, hs]
                        if f32_out:
                            x_in = x_in.bitcast(F32)
                        eng.tensor_sub(t1, x_in, mbt)
                        if trivial_gb:
                            eng.tensor_mul(o, t1, rbt)
                        else:
                            eng.tensor_mul(t1, t1, rbt)
                            eng.tensor_scalar(
                                o, t1, lngb_sb[:, ln_idx, 0, et],
                                lngb_sb[:, ln_idx, 1, et], ALU.mult, ALU.add)
                        if post_et is not None:
                            post_et(et, hs)

            # ---------------- GLRU ----------------
            CH = [(0, 512), (512, SL - 512)]
            x1 = cp.tile([128, 4, WT], BF16, tag="x1")
            with tc.tile_pool(name="psg", bufs=6, space="PSUM") as psg:
                # gv2 first; kk-outer on the 512-chunk so the first matmuls
                # need only the first xt/winT DMA chunks
                gts = [psg.tile([128, 512], F32, tag="mm", name=f"gv{et}")
                       for et in range(4)]
                for kk in range(4):
                    for et in range(4):
                        nc.tensor.matmul(
                            gts[et], winT_sb[:, kk, D + et * 128:D + (et + 1) * 128],
                            xt_sb[:, kk, 0:512], start=kk == 0, stop=kk == 3)
                for et in range(4):
                    nc.vector.tensor_copy(gv2_sb[:, et, 0:512], gts[et])
                for et in range(4):
                    g = psg.tile([128, SL - 512], F32, tag="mm")
                    for kk in range(4):
                        nc.tensor.matmul(
                            g, winT_sb[:, kk, D + et * 128:D + (et + 1) * 128],
                            xt_sb[:, kk, 512:SL], start=kk == 0, stop=kk == 3)
                    nc.vector.tensor_copy(gv2_sb[:, et, 512:SL], g)
                for et in range(4):                     # u, cand, w, scan per et
                    for c0, cn in CH:
                        g = psg.tile([128, cn], F32, tag="mm")
                        for kk in range(4):
                            nc.tensor.matmul(
                                g, winT_sb[:, kk, et * 128:(et + 1) * 128],
                                xt_sb[:, kk, c0:c0 + cn], start=kk == 0, stop=kk == 3)
                        nc.scalar.activation(u_sb[:, et, c0:c0 + cn], g, AF.Sigmoid)
                    for c0, cn in CH:
                        cd = psg.tile([128, cn], F32, tag="mm")
                        for kk in range(4):
                            nc.tensor.matmul(
                                cd, wstT_sb[:, kk, et * 128:(et + 1) * 128],
                                gv2_sb[:, kk, c0:c0 + cn], start=kk == 0, stop=kk == 3)
                        nc.vector.scalar_tensor_tensor(
                            w_sb[:, et, c0:c0 + cn], u_sb[:, et, c0:c0 + cn], 1.0,
                            cd, ALU.subtract, ALU.mult)
                    nc.vector.tensor_tensor_scan(
                        y_sb[:, et, :], u_sb[:, et, :], w_sb[:, et, :], 0.0,
                        ALU.mult, ALU.add)
                    if et == 3:
                        # pre-warm the sqrt act table (anchored post-sigmoid)
                        warm = rw.tile([1, 1], F32, tag="warm", name="warm1")
                        nc.scalar.activation(warm, u_sb[0:1, 3, SL - 1:SL],
                                             AF.Sqrt, bias=1.0, scale=0.0)

                # ---- LN1: x1 = LN(y[WU:WU+WT]) ----
                layer_norm(lambda et: y_sb[:, et, WU:WU + WT], WT,
                           lambda et: x1[:, et, :], 0)

                # ---- q/k/v ----
                q_sb = cp.tile([128, 4, TL], BF16, tag="q")
                k_sb = cp.tile([128, 4, WT], BF16, tag="k")
                for h in range(4):
                    qp = psg.tile([128, TL], F32, tag="mm")
                    for kk in range(4):
                        nc.tensor.matmul(qp, wqk_sb[:, kk, h * 128:(h + 1) * 128],
                                         x1[:, kk, HALO:HALO + TL],
                                         start=kk == 0, stop=kk == 3)
                    nc.vector.tensor_copy(q_sb[:, h, :], qp)
                    kp = psg.tile([128, 512], F32, tag="mm")
                    kp2 = pq.tile([128, 32], F32, tag="sc")
                    for kk in range(4):
                        nc.tensor.matmul(kp, wqk_sb[:, 4 + kk, h * 128:(h + 1) * 128],
                                         x1[:, kk, 0:512], start=kk == 0, stop=kk == 3)
                        nc.tensor.matmul(kp2, wqk_sb[:, 4 + kk, h * 128:(h + 1) * 128],
                                         x1[:, kk, 512:WT], start=kk == 0, stop=kk == 3)
                    nc.scalar.activation(k_sb[:, h, 0:512], kp, AF.Copy)
                    nc.vector.tensor_copy(k_sb[:, h, 512:WT], kp2)

                # v token-major, one 128-row window per attention chunk
                v_sb = cp.tile([128, NCH, D], BF16, tag="v")
                for c in range(NCH):
                    rows = min(128, WT - 96 * c)
                    vp = psg.tile([128, D], F32, tag="mm")
                    for kk in range(4):
                        nc.tensor.matmul(vp[0:rows, :],
                                         x1[:, kk, 96 * c:96 * c + rows],
                                         wvp_sb[:, kk, :],
                                         start=kk == 0, stop=kk == 3)
                    eng_v = nc.vector if c % 2 else nc.scalar
                    if c % 2:
                        nc.vector.tensor_copy(v_sb[0:rows, c, :], vp[0:rows, :])
                    else:
                        nc.scalar.activation(v_sb[0:rows, c, :], vp[0:rows, :],
                                             AF.Copy)

            # ---------------- banded attention ----------------
            # phase 1: all scores -> e; one 512-wide PSUM tile per head
            warm_e = rw.tile([1, 1], F32, tag="warm", name="warm_exp")
            nc.scalar.activation(warm_e, x1[0:1, 1, 0:1], AF.Exp, bias=1.0, scale=0.0)
            e_all = cp.tile([128, 4, 512], BF16, tag="eall")
            for h in range(4):
                sc = pq.tile([128, 512], F32, tag="sc")
                for c in range(5):
                    nc.tensor.matmul(sc[:, 96 * c:96 * c + 96],
                                     k_sb[:, h, 96 * c:96 * c + 128],
                                     q_sb[:, h, 96 * c:96 * c + 96],
                                     start=True, stop=True)
                nc.tensor.matmul(sc[0:64, 480:512], k_sb[:, h, 480:WT],
                                 q_sb[:, h, 480:TL], start=True, stop=True)
                nc.tensor.matmul(sc[64:128, 480:512], wtile[:, 0:64],
                                 wtile[:, 0:32], start=True, stop=True)
                nc.scalar.activation(e_all[:, h, :], sc, AF.Exp, scale=SCL)
                nc.gpsimd.tensor_mul(e_all[:, h, :], e_all[:, h, :], mask_sb)
                if h == 3:
                    warm3 = rw.tile([1, 1], F32, tag="warm", name="warm3")
                    nc.scalar.activation(warm3, e_all[0:1, 3, 0:1], AF.Sqrt,
                                         bias=1.0, scale=0.0)

            # phase 2: den + ao per head, proj accumulation interleaved
            a2 = cp.tile([128, 4, TL], BF16, tag="a2")
            x2pre = cp.tile([128, 4, TL], BF16, tag="x2pre")
            x2 = cp.tile([128, 4, TL], BF16, tag="x2")
            x3 = cp.tile([128, 4, TL], F32R, tag="x3")
            outt = cp.tile([128, 4, TL], F32, tag="outt")
            with tc.tile_pool(name="psf", bufs=4, space="PSUM") as psf:
                proj = [psf.tile([128, TL], F32, tag="mm", name=f"proj{i}")
                        for i in range(4)]
                with tc.tile_pool(name="pa", bufs=2, space="PSUM") as pa:
                    for h in range(4):
                        hsl = slice(h * 128, (h + 1) * 128)
                        den = pq.tile([1, TL], F32, tag="sc", name=f"den{h}")
                        ao = pa.tile([128, TL], F32, tag="ao")
                        nc.tensor.matmul(den[0:1, 0:480], ones16,
                                         e_all[:, h, 0:480], start=True, stop=True)
                        nc.tensor.matmul(den[0:1, 480:TL], ones16[0:64, :],
                                         e_all[0:64, h, 480:512], start=True, stop=True)
                        for c in range(NCH):
                            tn = 96 if c < 5 else TL - 480
                            kn = min(128, WT - 96 * c)
                            nc.tensor.matmul(ao[:, 96 * c:96 * c + tn],
                                             v_sb[0:kn, c, hsl],
                                             e_all[0:kn, h, 96 * c:96 * c + tn],
                                             start=True, stop=True)
                        rec = rw.tile([1, TL], F32, tag="rec")
                        nc.vector.reciprocal(rec, den)
                        dbc = bcp.tile([128, TL], F32, tag="dbc")
                        nc.gpsimd.partition_broadcast(dbc, rec)
                        nc.vector.tensor_mul(a2[:, h, :], ao, dbc)
                        for et in range(4):
                            nc.tensor.matmul(proj[et],
                                             wvp_sb[:, 4 + h, et * 128:(et + 1) * 128],
                                             a2[:, h, :], start=h == 0, stop=False)
                for et in range(4):
                    nc.tensor.matmul(proj[et], ident_sb,
                                     x1[:, et, HALO:HALO + TL],
                                     start=False, stop=True)
                    if et % 2:
                        nc.vector.tensor_copy(x2pre[:, et, :], proj[et])
                    else:
                        nc.scalar.activation(x2pre[:, et, :], proj[et], AF.Copy)
                layer_norm(lambda et: x2pre[:, et, :], TL,
                           lambda et: x2[:, et, :], 1)

                if trivial_gb:
                    xf = x2
                else:
                    xf = cp.tile([128, 4, TL], BF16, tag="xf")
                    layer_norm(lambda et: x2[:, et, :], TL,
                               lambda et: xf[:, et, :], 2)

                warm_g = rw.tile([1, 1], F32, tag="warm", name="warm_gelu")
                nc.scalar.activation(warm_g, x2[0:1, 1, 0:1], AF.Gelu, bias=1.0, scale=0.0)
                hg = cp.tile([128, 16, TL], BF16, tag="hg")
                for ft in range(16):
                    hp = psf.tile([128, TL], F32, tag="mm")
                    for kk in range(4):
                        nc.tensor.matmul(hp, w1all[:, kk, ft * 128:(ft + 1) * 128],
                                         xf[:, kk, :], start=kk == 0, stop=kk == 3)
                    nc.scalar.activation(hg[:, ft, :], hp, AF.Gelu,
                                         bias=b1_sb[:, ft:ft + 1])
                    if ft == 15:
                        warm2 = rw.tile([1, 1], F32, tag="warm", name="warm2")
                        nc.scalar.activation(warm2, hg[0:1, 15, 0:1], AF.Sqrt, bias=1.0, scale=0.0)

                # FFN2 et-outer; LN4 stats emitted right after each x3-et so
                # only et3's stats trail the last accumulation
                for et in range(4):
                    op = psf.tile([128, TL], F32, tag="mm")
                    for kk in range(16):
                        nc.tensor.matmul(op, w2all[:, kk, et * 128:(et + 1) * 128],
                                         hg[:, kk, :], start=kk == 0, stop=kk == 15)
                    nc.vector.scalar_tensor_tensor(
                        x3[:, et, :], op, b2_sb[:, et:et + 1],
                        x2[:, et, :], ALU.add, ALU.add)

                # ---- LN4 -> output (column-split tail) ----
                layer_norm(lambda et: x3[:, et, :], TL,
                           lambda et: outt[:, et, :], 3, col_splits=2,
                           post_et=lambda et, hs: (nc.scalar if et % 2 else nc.sync)
                           .dma_start(out_d[et, :, hs], outt[:, et, hs]),
                           f32_out=True)

    nc.compile()
    return nc


def _host_inputs(x, W_in, W_state, glru_g, glru_b, Wq, Wk, Wv, Wp, attn_g,
                 attn_b, ffn_g, ffn_b, W1, b1, W2, b2, out_g, out_b):
    f32, bf = np.float32, ml_dtypes.bfloat16
    cb = lambda a: np.ascontiguousarray(np.asarray(a, f32).astype(bf))
    lngb = np.zeros((128, 4, 2, 4), f32)
    for li, (g, b) in enumerate([(glru_g, glru_b), (attn_g, attn_b),
                                 (ffn_g, ffn_b), (out_g, out_b)]):
        lngb[:, li, 0, :] = np.asarray(g, f32).reshape(4, 128).T
        lngb[:, li, 1, :] = np.asarray(b, f32).reshape(4, 128).T
    shared = {
        "w_inT": cb(np.asarray(W_in).T),
        "w_stateT": cb(-np.asarray(W_state).T),
        "wqkT": cb(np.concatenate([np.asarray(Wq).T, np.asarray(Wk).T], 1)),
        "wvpT": cb(np.concatenate([np.asarray(Wv).T, np.asarray(Wp).T], 1)),
        "w1T": cb(np.asarray(W1).T), "w2T": cb(np.asarray(W2).T),
        "lngb": lngb,
        "b1c": np.ascontiguousarray(np.asarray(b1, f32).reshape(FFD // 128, 128).T),
        "b2c": np.ascontiguousarray(np.asarray(b2, f32).reshape(D // 128, 128).T),
        "onesb": np.broadcast_to(np.array([1.0, 1.0 / D], f32), (128, 2))
                  .astype(bf).copy(),
        "ones32": np.full((128, 1), 1.0 / D, f32),
        "onesr": np.ones((1, 128), f32),
        "ident": np.eye(128, dtype=f32).astype(bf),
    }
    xpad = np.zeros((B, T + WU + HALO + HALO, D), f32)
    xpad[:, WU + HALO:WU + HALO + T] = np.asarray(x, f32)
    in_maps = []
    for core in range(NCORES):
        b, half = core // 2, core % 2
        h0 = half * TL
        m = dict(shared)
        m["xt"] = cb(xpad[b, h0:h0 + SL].T)
        mask = np.zeros((128, 512), f32)
        ii = np.arange(128)[:, None]
        for c in range(NCH):
            tn = 96 if c < 5 else TL - 480
            kn = min(128, WT - 96 * c)
            jj = np.arange(tn)[None, :]
            tp = h0 + 96 * c + ii - HALO        # absolute t' token
            band = (np.abs(ii - jj - HALO) <= WIN) & (tp >= 0) & (tp < T) \
                & (ii < kn)
            mask[:, 96 * c:96 * c + tn] = band
        m["maskc"] = mask.astype(bf)
        in_maps.append(m)
    return in_maps


def kernel(**inputs):
    trivial = all(
        bool(np.all(np.asarray(inputs[g]) == 1.0)) for g in
        ("glru_g", "attn_g", "ffn_g", "out_g")
    ) and all(
        bool(np.all(np.asarray(inputs[b]) == 0.0)) for b in
        ("glru_b", "attn_b", "ffn_b", "out_b")
    )
    key = f"nc_{trivial}"
    if key not in _CACHE:
        _CACHE[key] = _build_nc(trivial_gb=trivial)
    nc = _CACHE[key]
    in_maps = _host_inputs(**inputs)
    res = run_bass_kernel_spmd(nc, in_maps, core_ids=list(range(NCORES)),
                               **_CACHE.get("run_kwargs", {}))
    _CACHE["last_result"] = res
    out = np.empty((B, T, D), np.float32)
    for core in range(NCORES):
        b, half = core // 2, core % 2
        o = np.asarray(res.results[core]["outp"]).astype(np.float32)
        out[b, half * TL:(half + 1) * TL, :] = o.reshape(D, TL).T
    return out
